# revision 60
# baseline (speedup 1.0000x reference)
"""GAT 2-layer message-passing network on 8 TRN2 NeuronCores (Bass/Tile).

v3: gather-free L1 via host-side edge-slot materialization of x.

Strategy (dst-sharded, dense slot layout):
 - Host: add self loops, sort edges by dst, shard dst-node ranges across
   cores (core c owns nodes [c*NPC, (c+1)*NPC) and ALL edges into them).
 - Slots: per superblock of SBG=3 dst-blocks, each core's edges packed
   DENSELY into Tsb = ceil(max_core_count/128) tiles (common layout across
   cores; only trailing padding). A tile touches at most 2 dst blocks
   (maxU=2): host assigns each slot to one-hot set A or B.
 - L1 (Phase B): host pre-gathers x rows into slot order (xgaT [165, SLOTS]
   bf16). Device: h_slot = xga @ [W1|Wsrc] per tile (PE), a_dst via
   oT-matmul against a local per-block table, ex = exp(lrelu(asrc+adst)),
   msg = [h*ex | ex], scatter-add per block via one-hot matmuls, normalize,
   +b1, relu, then h2 = o1 @ [W2|Wsrc2|Wdst2] -> h2c [P, NB*4].
 - AllGather h2loc [NPC,4] f32 -> h2tab [N,4] f32 (= gather table viewed as
   [N/16, 64]: 16 nodes per 256B row; NO repack needed).
 - L2 (Phase C): same slots: dma_gather h2tab16 rows by src//16 (single
   chunk, int16), on-chip lane extract by src%16, ex2 via oT2/adst2 table,
   4-wide messages, one-hot scatter, normalize; batched log_softmax at end.
"""
import sys

if "/opt/trn_rl_repo" not in sys.path:
    sys.path.insert(0, "/opt/trn_rl_repo")

import math
import numpy as np
import ml_dtypes

import concourse.bass as bass
import concourse.bacc as bacc
import concourse.mybir as mybir
import concourse.tile as tile
from concourse import bass_utils

P = 128
NEG = 0.2
NQUEUE = 4
SBG = 3
MAXT = 7

# Tile's DMASW sem-lane assignment round-robins over all Pool DMAs, which
# breaks the per-lane FIFO assumption when SWDGE DMAs run on multiple queues
# (out-of-order completion across queues under one counting sem). Patch the
# lane choice to lane == queue_num: per-lane FIFO again holds (each HW ring
# drains in order), and queues get independent lanes.
from concourse import tile_sem_assignment as _tsa  # noqa: E402

if not getattr(_tsa.TileClockTick, "_qaware_patched", False):
    _orig_assign_tick = _tsa.TileClockTick._assign_tick

    def _qaware_assign_tick(self, inst):
        if (isinstance(inst, _tsa.DMAInst)
                and inst.engine == mybir.EngineType.Pool):
            self.next_sw_dma_idx = getattr(inst, "queue_num", 0) or 0
        return _orig_assign_tick(self, inst)

    _tsa.TileClockTick._assign_tick = _qaware_assign_tick
    _tsa.TileClockTick._qaware_patched = True


def _wrap16(flat):
    """[n] -> [128, n//16] wrapped in 16 partitions, replicated x8."""
    w = flat.reshape(-1, 16).T
    return np.tile(w, (8, 1))


# ----------------------------------------------------------------------------
# host-side data prep
# ----------------------------------------------------------------------------

def prep(inputs, cfg):
    N, F, H, C, CLS, NC = cfg["N"], cfg["F"], cfg["H"], cfg["C"], cfg["CLS"], cfg["NC"]
    x = np.asarray(inputs["x"], np.float32)
    ei = np.asarray(inputs["edge_index"])
    W1 = np.asarray(inputs["W1"], np.float32)
    as1 = np.asarray(inputs["att_src1"], np.float32)
    ad1 = np.asarray(inputs["att_dst1"], np.float32)
    b1 = np.asarray(inputs["b1"], np.float32)
    W2 = np.asarray(inputs["W2"], np.float32)
    as2 = np.asarray(inputs["att_src2"], np.float32)
    ad2 = np.asarray(inputs["att_dst2"], np.float32)
    b2 = np.asarray(inputs["b2"], np.float32)

    HC = H * C                          # 256
    R1 = HC + H                         # 260 = [h | asrc]
    NPC = N // NC
    NB = math.ceil(NPC / P)
    NPCp = NB * P
    bf16 = ml_dtypes.bfloat16
    f8 = ml_dtypes.float8_e4m3
    KH = 96                             # DoubleRow half-contraction (2*96 >= F)

    # ---- weights / constants -------------------------------------------------
    W1r = W1.reshape(F, H, C)
    Wsrc = np.einsum("fhc,hc->fh", W1r, as1)
    Wdst = np.einsum("fhc,hc->fh", W1r, ad1)
    W1s = np.concatenate([W1, Wsrc], axis=1).astype(bf16)     # [F, R1]
    Wd = Wdst.astype(bf16)                                    # [F, H]
    Wsrc2 = W2 @ as2.reshape(CLS, 1)
    Wdst2 = W2 @ ad2.reshape(CLS, 1)
    W2aug = np.concatenate([W2, Wsrc2, Wdst2], axis=1).astype(bf16)  # [HC, 4]

    b1rep = np.tile(b1[None, :], (P, 1)).astype(bf16)
    b2rep = np.tile(b2[None, :], (P, 1)).astype(np.float32)
    ident = np.eye(P, dtype=bf16)

    # ---- edges ---------------------------------------------------------------
    src_all = np.concatenate([ei[0], np.arange(N, dtype=ei.dtype)]).astype(np.int64)
    dst_all = np.concatenate([ei[1], np.arange(N, dtype=ei.dtype)]).astype(np.int64)
    order = np.argsort(dst_all, kind="stable")
    src_s = src_all[order]
    dst_s = dst_all[order]

    # per (core, block) counts
    cnt = np.zeros((NC, NB), np.int64)
    for c in range(NC):
        for b in range(NB):
            base = c * NPC + b * P
            hi = min(base + P, (c + 1) * NPC)
            cnt[c, b] = (np.searchsorted(dst_s, hi) -
                         np.searchsorted(dst_s, base))

    # superblock metadata (common across cores)
    sbs = []
    tile_base = 0
    nsb = math.ceil(NB / SBG)
    for s in range(nsb):
        blo, bhi = s * SBG, min((s + 1) * SBG, NB)
        persb = cnt[:, blo:bhi]
        Tsb = int(np.ceil(persb.sum(axis=1).max() / P))
        t0 = np.zeros(bhi - blo, np.int64)
        t1 = np.zeros(bhi - blo, np.int64)
        for bi in range(bhi - blo):
            lo = persb[:, :bi].sum(axis=1)
            hi2 = persb[:, :bi + 1].sum(axis=1)
            t0[bi] = lo.min() // P
            t1[bi] = min(int(np.ceil(hi2 / P).max()) - 1, Tsb - 1)
        # per tile: ordered union of touched blocks (local block index)
        uA = np.full(Tsb, -1, np.int64)
        uB = np.full(Tsb, -1, np.int64)
        for t in range(Tsb):
            u = [bi for bi in range(bhi - blo) if t0[bi] <= t <= t1[bi]]
            assert 1 <= len(u) <= 2, (s, t, u)
            uA[t] = u[0]
            if len(u) == 2:
                uB[t] = u[1]
        sbs.append(dict(base=tile_base, S=Tsb, blo=blo, bhi=bhi,
                        t0=t0, t1=t1, uA=uA, uB=uB))
        tile_base += Tsb
    Tsum = tile_base
    SLOT = Tsum * P
    S_MAX = max(sb["S"] for sb in sbs)

    # ---- per-core slot arrays ------------------------------------------------
    x_bf = x.astype(bf16)
    in_maps = []
    shared = {
        "w1s_a": np.ascontiguousarray(W1s[:P]),
        "w1s_b": np.ascontiguousarray(W1s[P:]),
        "wd_a": np.ascontiguousarray(Wd[:P]),
        "wd_b": np.ascontiguousarray(Wd[P:]),
        "w2aug": W2aug, "b1rep": b1rep, "b2rep": b2rep, "ident": ident,
    }
    iotaP = np.arange(P)
    for c in range(NC):
        srcslot = np.zeros(SLOT, np.int64)
        dlocA = np.full(SLOT, 255, np.int64)
        dlocB = np.full(SLOT, 255, np.int64)
        core_lo = np.searchsorted(dst_s, c * NPC)
        core_hi = np.searchsorted(dst_s, (c + 1) * NPC)
        cs = src_s[core_lo:core_hi]
        cd = dst_s[core_lo:core_hi]
        for sb in sbs:
            base_n = c * NPC + sb["blo"] * P
            hi_n = min(c * NPC + sb["bhi"] * P, (c + 1) * NPC)
            lo_i = np.searchsorted(cd, base_n)
            hi_i = np.searchsorted(cd, hi_n)
            es = cs[lo_i:hi_i]
            ed = cd[lo_i:hi_i]
            n = len(es)
            assert n <= sb["S"] * P
            s0 = sb["base"] * P
            srcslot[s0:s0 + n] = es
            # block-local dst and A/B role per slot
            pos = np.arange(n)
            trel = pos // P
            bloc = (ed - c * NPC) // P - sb["blo"]          # local block idx
            dv = ed - c * NPC - (bloc + sb["blo"]) * P
            isA = sb["uA"][trel] == bloc
            isB = sb["uB"][trel] == bloc
            assert np.all(isA | isB), (c, sb["base"])
            dlocA[s0 + pos[isA]] = dv[isA]
            dlocB[s0 + pos[isB]] = dv[isB]
        m = dict(shared)
        xgb = np.ascontiguousarray(x_bf[srcslot].T)         # [F, SLOT] bf16
        m["xga_a"] = np.ascontiguousarray(xgb[:P])
        m["xga_b"] = np.ascontiguousarray(xgb[P:])
        xl = np.zeros((F, NPCp), dtype=bf16)
        xl[:, :NPC] = x_bf[c * NPC:(c + 1) * NPC].T
        m["xtl_a"] = np.ascontiguousarray(xl[:P])
        m["xtl_b"] = np.ascontiguousarray(xl[P:])
        # h2 gather idx: table = [NC*P, NB*4] f32 (partition-major per core)
        sc = srcslot // NPC
        loc = srcslot % NPC
        flat = sc * (P * NB) + (loc % P) * NB + loc // P    # row-of-4 index
        m["ihsrc16"] = _wrap16((flat // 16).astype(np.int16))
        lane = flat % 16
        # host-prebuilt one-hots and lane masks
        dA2 = dlocA.reshape(Tsum, P)                        # [t, s]
        dB2 = dlocB.reshape(Tsum, P)
        ohA = (dA2[:, :, None] == iotaP[None, None, :])     # [t, s, d]
        ohB = (dB2[:, :, None] == iotaP[None, None, :])
        m["ohA"] = ohA.transpose(1, 0, 2).reshape(P, Tsum * P).astype(bf16)
        m["ohB"] = ohB.transpose(1, 0, 2).reshape(P, Tsum * P).astype(bf16)
        m["oTA"] = ohA.transpose(2, 0, 1).reshape(P, Tsum * P).astype(bf16)
        m["oTB"] = ohB.transpose(2, 0, 1).reshape(P, Tsum * P).astype(bf16)
        msk = (lane.reshape(Tsum, P)[:, :, None] ==
               np.arange(16)[None, None, :])                # [t, s, 16]
        m["msk"] = msk.transpose(1, 0, 2).reshape(P, Tsum * 16).astype(bf16)
        in_maps.append(m)

    meta = dict(cfg, R1=R1, HC=HC, NPC=NPC, NPCp=NPCp, NB=NB, Tsum=Tsum,
                SLOT=SLOT, S_MAX=S_MAX, sbs=sbs)
    return in_maps, meta


# ----------------------------------------------------------------------------
# device program
# ----------------------------------------------------------------------------

def _sub(ap, elem_off, dims):
    return bass.AP(ap.tensor, ap.offset + elem_off, [ap.ap[0], *list(dims)])


def build(meta, nc=None):
    N, F, H, C, CLS = meta["N"], meta["F"], meta["H"], meta["C"], meta["CLS"]
    NC, R1, HC = meta["NC"], meta["R1"], meta["HC"]
    NPC, NPCp, NB = meta["NPC"], meta["NPCp"], meta["NB"]
    Tsum, SLOT = meta["Tsum"], meta["SLOT"]
    sbs = meta["sbs"]
    FB = F - P                           # 37
    NT16 = N // 16                       # h2tab rows of 16 nodes
    RL2 = 64                             # f32 elems per 256B gather row

    f32, bf16, i16 = mybir.dt.float32, mybir.dt.bfloat16, mybir.dt.int16
    f8 = mybir.dt.float8e4
    KH = 96
    DR = mybir.MatmulPerfMode.DoubleRow
    EXP = mybir.ActivationFunctionType.Exp
    LN = mybir.ActivationFunctionType.Ln

    if nc is None:
        nc = bacc.Bacc("TRN2", target_bir_lowering=False, debug=False,
                       num_devices=NC, num_swdge_queues=NQUEUE)

    qrr = [0]

    def gather_split(out_tile, rel, segT, elem, table, ix_tile):
        done = 0
        while done < segT:
            tt = min(MAXT, segT - done)
            r = rel + done
            nc.gpsimd.dma_gather(
                bass.AP(out_tile[:].tensor, out_tile[:].offset + r * elem,
                        [out_tile[:].ap[0], [elem, tt], [1, elem]]),
                table,
                ix_tile[:, r * 8:(r + tt) * 8],
                tt * P, tt * P, elem,
                queue_num=qrr[0] % NQUEUE,
            )
            qrr[0] += 1
            done += tt

    xga_a_d = nc.dram_tensor("xga_a", [P, SLOT], bf16, kind="ExternalInput")
    xga_b_d = nc.dram_tensor("xga_b", [FB, SLOT], bf16, kind="ExternalInput")
    xtl_a_d = nc.dram_tensor("xtl_a", [P, NPCp], bf16, kind="ExternalInput")
    xtl_b_d = nc.dram_tensor("xtl_b", [FB, NPCp], bf16, kind="ExternalInput")
    w1s_a_d = nc.dram_tensor("w1s_a", [P, R1], bf16, kind="ExternalInput")
    w1s_b_d = nc.dram_tensor("w1s_b", [FB, R1], bf16, kind="ExternalInput")
    wd_a_d = nc.dram_tensor("wd_a", [P, H], bf16, kind="ExternalInput")
    wd_b_d = nc.dram_tensor("wd_b", [FB, H], bf16, kind="ExternalInput")
    w2aug_d = nc.dram_tensor("w2aug", [HC, 4], bf16, kind="ExternalInput")
    b1rep_d = nc.dram_tensor("b1rep", [P, HC], bf16, kind="ExternalInput")
    b2rep_d = nc.dram_tensor("b2rep", [P, CLS], f32, kind="ExternalInput")
    ident_d = nc.dram_tensor("ident", [P, P], bf16, kind="ExternalInput")
    ihsrc_d = nc.dram_tensor("ihsrc16", [P, Tsum * 8], i16, kind="ExternalInput")
    ohA_d = nc.dram_tensor("ohA", [P, SLOT], bf16, kind="ExternalInput")
    ohB_d = nc.dram_tensor("ohB", [P, SLOT], bf16, kind="ExternalInput")
    oTA_d = nc.dram_tensor("oTA", [P, SLOT], bf16, kind="ExternalInput")
    oTB_d = nc.dram_tensor("oTB", [P, SLOT], bf16, kind="ExternalInput")
    msk_d = nc.dram_tensor("msk", [P, Tsum * 16], bf16, kind="ExternalInput")
    out_d = nc.dram_tensor("out", [P, NB * CLS], f32, kind="ExternalOutput")

    h2locp = nc.dram_tensor("h2locp", [P, NB * 4], f32, kind="Internal")
    h2tabp = nc.dram_tensor("h2tabp", [NC * P, NB * 4], f32, kind="Internal",
                            addr_space="Shared" if NC > 4 else "Local")

    with tile.TileContext(nc) as tc:
        with tc.tile_pool(name="const", bufs=1) as cp:
            w1sa = cp.tile([P, R1], bf16)
            nc.sync.dma_start(out=w1sa[:], in_=w1s_a_d[:, :])
            w1sb = cp.tile([FB, R1], bf16)
            nc.sync.dma_start(out=w1sb[:], in_=w1s_b_d[:, :])
            w2s = []
            for k in range(2):
                w2k = cp.tile([P, 4], bf16, name=f"w2k{k}")
                nc.sync.dma_start(out=w2k[:], in_=w2aug_d[k * P:(k + 1) * P, :])
                w2s.append(w2k)
            b1s = cp.tile([P, HC], bf16)
            nc.sync.dma_start(out=b1s[:], in_=b1rep_d[:, :])
            b2s = cp.tile([P, CLS], f32)
            nc.sync.dma_start(out=b2s[:], in_=b2rep_d[:, :])
            idn = cp.tile([P, P], bf16)
            nc.sync.dma_start(out=idn[:], in_=ident_d[:, :])
            ixs = cp.tile([P, Tsum * 8], i16)
            adw = cp.tile([P, NB * H], bf16)      # a_dst1 per local node
            ad2w = cp.tile([P, NB], bf16)         # a_dst2 per local node
            h2c = cp.tile([P, NB * 4], f32)       # h2 collect [out2|asrc2|adst2]
            vcol = cp.tile([P, NB * CLS], f32)    # L2 logits collect

            # ---------------- Phase A-mini: local a_dst1 table ---------------
            with tc.tile_pool(name="pa", bufs=1) as pa, \
                 tc.tile_pool(name="psa", bufs=4, space="PSUM") as psa:
                xla = pa.tile([P, NPCp], bf16)
                nc.sync.dma_start(out=xla[:], in_=xtl_a_d[:, :])
                xlb = pa.tile([FB, NPCp], bf16)
                nc.sync.dma_start(out=xlb[:], in_=xtl_b_d[:, :])
                wda = pa.tile([P, H], bf16)
                nc.sync.dma_start(out=wda[:], in_=wd_a_d[:, :])
                wdb = pa.tile([FB, H], bf16)
                nc.sync.dma_start(out=wdb[:], in_=wd_b_d[:, :])
                for nt in range(NB):
                    ps = psa.tile([P, H], f32, tag="ps")
                    nc.tensor.matmul(out=ps[:], lhsT=xla[:, nt * P:(nt + 1) * P],
                                     rhs=wda[:], start=True, stop=False)
                    nc.tensor.matmul(out=ps[:], lhsT=xlb[:, nt * P:(nt + 1) * P],
                                     rhs=wdb[:], start=False, stop=True)
                    nc.vector.tensor_copy(out=adw[:, nt * H:(nt + 1) * H],
                                          in_=ps[:])

            # ---------------- Phase B: L1 edge pass --------------------------
            with tc.tile_pool(name="pbg", bufs=3) as pbg, \
                 tc.tile_pool(name="pbo", bufs=2) as pbo, \
                 tc.tile_pool(name="pbb", bufs=2) as pbb, \
                 tc.tile_pool(name="psh", bufs=2, space="PSUM") as psh, \
                 tc.tile_pool(name="psk", bufs=1, space="PSUM") as psk, \
                 tc.tile_pool(name="psb", bufs=3, space="PSUM") as psb, \
                 tc.tile_pool(name="pst", bufs=1, space="PSUM") as pst, \
                 tc.tile_pool(name="ps2", bufs=1, space="PSUM") as ps2p:
                for sb in sbs:
                    base, S = sb["base"], sb["S"]
                    blo = sb["blo"]
                    nblk = sb["bhi"] - blo
                    has_b = bool((sb["uB"] >= 0).any())
                    xa = pbg.tile([P, S * P], bf16, tag="xa")
                    nc.sync.dma_start(out=xa[:],
                                      in_=xga_a_d[:, base * P:(base + S) * P])
                    xb = pbg.tile([FB, S * P], bf16, tag="xb")
                    nc.sync.dma_start(out=xb[:],
                                      in_=xga_b_d[:, base * P:(base + S) * P])
                    # one-hots (host-prebuilt)
                    oTA = pbo.tile([P, S * P], bf16, tag="oTA")
                    nc.sync.dma_start(out=oTA[:],
                                      in_=oTA_d[:, base * P:(base + S) * P])
                    ohA = pbo.tile([P, S * P], bf16, tag="ohA")
                    nc.sync.dma_start(out=ohA[:],
                                      in_=ohA_d[:, base * P:(base + S) * P])
                    if has_b:
                        oTB = pbo.tile([P, S * P], bf16, tag="oTB")
                        nc.sync.dma_start(out=oTB[:],
                                          in_=oTB_d[:, base * P:(base + S) * P])
                        ohB = pbo.tile([P, S * P], bf16, tag="ohB")
                        nc.sync.dma_start(out=ohB[:],
                                          in_=ohB_d[:, base * P:(base + S) * P])
                    # h per slot -> psum -> g (bf16)
                    g = pbg.tile([P, S * R1], bf16, tag="g")
                    for t in range(S):
                        ph = psh.tile([P, R1], f32, tag="ph")
                        nc.tensor.matmul(out=ph[:], lhsT=xa[:, t * P:(t + 1) * P],
                                         rhs=w1sa[:], start=True, stop=False)
                        nc.tensor.matmul(out=ph[:], lhsT=xb[:, t * P:(t + 1) * P],
                                         rhs=w1sb[:], start=False, stop=True)
                        nc.scalar.copy(out=g[:, t * R1:(t + 1) * R1], in_=ph[:])
                    # a_dst per slot
                    pad = psk.tile([P, 512], f32, tag="stp")
                    for t in range(S):
                        bA = blo + int(sb["uA"][t])
                        bBl = int(sb["uB"][t])
                        nc.tensor.matmul(
                            out=pad[:, t * H:(t + 1) * H],
                            lhsT=oTA[:, t * P:(t + 1) * P],
                            rhs=adw[:, bA * H:(bA + 1) * H],
                            start=True, stop=(bBl < 0),
                            skip_group_check=True)
                        if bBl >= 0:
                            bB = blo + bBl
                            nc.tensor.matmul(
                                out=pad[:, t * H:(t + 1) * H],
                                lhsT=oTB[:, t * P:(t + 1) * P],
                                rhs=adw[:, bB * H:(bB + 1) * H],
                                start=False, stop=True,
                                skip_group_check=True)
                    # ex = exp(lrelu(asrc + adst))  [P, S*H] f32
                    ex = pbb.tile([P, S * H], f32, tag="ex")
                    nc.vector.tensor_tensor(
                        out=ex[:].rearrange("p (t h) -> p t h", t=S),
                        in0=_sub(g[:], HC, [[R1, S], [1, H]]),
                        in1=_sub(pad[:], 0, [[H, S], [1, H]]),
                        op=mybir.AluOpType.add)
                    tmp = pbb.tile([P, S * H], f32, tag="tmp")
                    nc.vector.tensor_scalar_mul(out=tmp[:], in0=ex[:], scalar1=NEG)
                    nc.vector.tensor_tensor(out=ex[:], in0=ex[:], in1=tmp[:],
                                            op=mybir.AluOpType.max)
                    nc.scalar.activation(out=ex[:], in_=ex[:], func=EXP)
                    # msg in place: h *= ex ; asrc cols := ex
                    nc.gpsimd.tensor_tensor(
                        out=_sub(g[:], 0, [[R1, S], [C, H], [1, C]]),
                        in0=_sub(g[:], 0, [[R1, S], [C, H], [1, C]]),
                        in1=_sub(ex[:], 0, [[H, S], [1, H], [0, C]]),
                        op=mybir.AluOpType.mult)
                    nc.vector.tensor_copy(
                        out=_sub(g[:], HC, [[R1, S], [1, H]]),
                        in_=ex[:].rearrange("p (t h) -> p t h", t=S))
                    # scatter-add per block + epilogue
                    for bi in range(nblk):
                        b = blo + bi
                        t0, t1 = int(sb["t0"][bi]), int(sb["t1"][bi])
                        pso = psb.tile([P, R1], f32, tag="pso")
                        for t in range(t0, t1 + 1):
                            oh = ohA if int(sb["uA"][t]) == bi else ohB
                            nc.tensor.matmul(
                                out=pso[:],
                                lhsT=oh[:, t * P:(t + 1) * P],
                                rhs=g[:, t * R1:t * R1 + R1],
                                start=(t == t0), stop=(t == t1))
                        den = pbb.tile([P, H], f32, tag="den")
                        nc.vector.tensor_scalar_max(out=den[:],
                                                    in0=pso[:, HC:HC + H],
                                                    scalar1=1e-20)
                        rde = pbb.tile([P, H], f32, tag="rde")
                        nc.vector.reciprocal(out=rde[:], in_=den[:])
                        o1 = pbb.tile([P, HC], bf16, tag="o1")
                        nc.vector.tensor_tensor(
                            out=_sub(o1[:], 0, [[C, H], [1, C]]),
                            in0=_sub(pso[:], 0, [[C, H], [1, C]]),
                            in1=_sub(rde[:], 0, [[1, H], [0, C]]),
                            op=mybir.AluOpType.mult)
                        nc.vector.tensor_tensor(out=o1[:], in0=o1[:], in1=b1s[:],
                                                op=mybir.AluOpType.add)
                        nc.vector.tensor_scalar_max(out=o1[:], in0=o1[:],
                                                    scalar1=0.0)
                        ph2 = ps2p.tile([P, 4], f32, tag="ph2")
                        for k in range(2):
                            ptr = pst.tile([P, P], bf16, tag="ptr")
                            nc.tensor.transpose(out=ptr[:],
                                                in_=o1[:, k * P:(k + 1) * P],
                                                identity=idn[:])
                            rT = pbb.tile([P, P], bf16, tag="rT")
                            nc.vector.tensor_copy(out=rT[:], in_=ptr[:])
                            nc.tensor.matmul(out=ph2[:], lhsT=rT[:],
                                             rhs=w2s[k][:],
                                             start=(k == 0), stop=(k == 1))
                        nc.vector.tensor_copy(out=h2c[:, b * 4:(b + 1) * 4],
                                              in_=ph2[:])
                        nc.vector.tensor_copy(out=ad2w[:, b:b + 1],
                                              in_=ph2[:, 3:4])

            # h2c -> h2locp DRAM (partition-major, one dense DMA)
            nc.sync.dma_start(out=h2locp[:, :], in_=h2c[:])

            # ---------------- AllGather ------------------------------------
            nc.gpsimd.collective_compute(
                "AllGather", mybir.AluOpType.bypass,
                replica_groups=[list(range(NC))],
                ins=[h2locp[:, :]], outs=[h2tabp[:, :]])
            NROW16 = NC * P * NB * 4 // RL2
            h2tab16 = bass.AP(h2tabp, 0, [[RL2, NROW16], [1, RL2]])

            # Phase-C gather index table (deferred out of the startup burst)
            nc.sync.dma_start(out=ixs[:], in_=ihsrc_d[:, :])

            # ---------------- Phase C: L2 edge pass --------------------------
            with tc.tile_pool(name="pcg", bufs=20) as pcg, \
                 tc.tile_pool(name="pco", bufs=3) as pco, \
                 tc.tile_pool(name="pcb", bufs=4) as pcb, \
                 tc.tile_pool(name="psk2", bufs=1, space="PSUM") as psk2, \
                 tc.tile_pool(name="psc", bufs=6, space="PSUM") as psc:
                for sb in sbs:
                    base, S = sb["base"], sb["S"]
                    blo = sb["blo"]
                    nblk = sb["bhi"] - blo
                    has_b = bool((sb["uB"] >= 0).any())
                    # gather + lane-extract per call: each dma_gather gets
                    # its own buffer so the 4 SWDGE queues generate
                    # descriptors concurrently (per-queue Q7 core pairs)
                    msk = pcb.tile([P, S * 16], bf16, tag="msk")
                    nc.sync.dma_start(out=msk[:],
                                      in_=msk_d[:, base * 16:(base + S) * 16])
                    m2p = pcb.tile([P, S * 4], f32, tag="m2p")
                    r = 0
                    while r < S:
                        tt = min(MAXT, S - r)
                        gc = pcg.tile([P, MAXT * RL2], f32, tag="gc")
                        nc.gpsimd.dma_gather(
                            bass.AP(gc[:].tensor, gc[:].offset,
                                    [gc[:].ap[0], [RL2, tt], [1, RL2]]),
                            h2tab16,
                            ixs[:, (base + r) * 8:(base + r + tt) * 8],
                            tt * P, tt * P, RL2,
                            queue_num=qrr[0] % NQUEUE,
                        )
                        qrr[0] += 1
                        tmc = pcb.tile([P, MAXT * RL2], bf16, tag="tmc")
                        nc.vector.tensor_tensor(
                            out=_sub(tmc[:], 0, [[RL2, tt], [16, 4], [1, 16]]),
                            in0=_sub(gc[:], 0, [[RL2, tt], [1, 4], [4, 16]]),
                            in1=_sub(msk[:], r * 16, [[16, tt], [0, 4], [1, 16]]),
                            op=mybir.AluOpType.mult)
                        nc.vector.tensor_reduce(
                            out=_sub(m2p[:], r * 4, [[4, tt], [1, 4]]),
                            in_=_sub(tmc[:], 0, [[RL2, tt], [16, 4], [1, 16]]),
                            axis=mybir.AxisListType.X,
                            op=mybir.AluOpType.add)
                        r += tt
                    # one-hots (host-prebuilt)
                    oTA = pco.tile([P, S * P], bf16, tag="oTA2")
                    nc.sync.dma_start(out=oTA[:],
                                      in_=oTA_d[:, base * P:(base + S) * P])
                    ohA = pco.tile([P, S * P], bf16, tag="ohA2")
                    nc.sync.dma_start(out=ohA[:],
                                      in_=ohA_d[:, base * P:(base + S) * P])
                    if has_b:
                        oTB = pco.tile([P, S * P], bf16, tag="oTB2")
                        nc.sync.dma_start(out=oTB[:],
                                          in_=oTB_d[:, base * P:(base + S) * P])
                        ohB = pco.tile([P, S * P], bf16, tag="ohB2")
                        nc.sync.dma_start(out=ohB[:],
                                          in_=ohB_d[:, base * P:(base + S) * P])
                    pad2 = psk2.tile([P, 512], f32, tag="stp2")
                    for t in range(S):
                        bA = blo + int(sb["uA"][t])
                        bBl = int(sb["uB"][t])
                        nc.tensor.matmul(
                            out=pad2[:, t:t + 1],
                            lhsT=oTA[:, t * P:(t + 1) * P],
                            rhs=ad2w[:, bA:bA + 1],
                            start=True, stop=(bBl < 0),
                            skip_group_check=True)
                        if bBl >= 0:
                            bB = blo + bBl
                            nc.tensor.matmul(
                                out=pad2[:, t:t + 1],
                                lhsT=oTB[:, t * P:(t + 1) * P],
                                rhs=ad2w[:, bB:bB + 1],
                                start=False, stop=True,
                                skip_group_check=True)
                    ex2 = pcb.tile([P, S], f32, tag="ex2")
                    nc.vector.tensor_tensor(
                        out=ex2[:],
                        in0=_sub(m2p[:], 2, [[4, S]]),
                        in1=_sub(pad2[:], 0, [[1, S]]),
                        op=mybir.AluOpType.add)
                    tm3 = pcb.tile([P, S], f32, tag="tm3")
                    nc.scalar.mul(out=tm3[:], in_=ex2[:], mul=NEG)
                    nc.vector.tensor_tensor(out=ex2[:], in0=ex2[:], in1=tm3[:],
                                            op=mybir.AluOpType.max)
                    nc.scalar.activation(out=ex2[:], in_=ex2[:], func=EXP)
                    # m2 = [h2_0*ex | h2_1*ex | ex | ex]  bf16
                    m2 = pcb.tile([P, S * 4], bf16, tag="m2")
                    nc.vector.tensor_tensor(
                        out=_sub(m2[:], 0, [[4, S], [1, CLS]]),
                        in0=_sub(m2p[:], 0, [[4, S], [1, CLS]]),
                        in1=_sub(ex2[:], 0, [[1, S], [0, CLS]]),
                        op=mybir.AluOpType.mult)
                    nc.scalar.copy(
                        out=_sub(m2[:], CLS, [[4, S], [1, 2]]),
                        in_=_sub(ex2[:], 0, [[1, S], [0, 2]]))
                    for bi in range(nblk):
                        b = blo + bi
                        t0, t1 = int(sb["t0"][bi]), int(sb["t1"][bi])
                        ps2 = psc.tile([P, 4], f32, tag="ps2")
                        for t in range(t0, t1 + 1):
                            oh = ohA if int(sb["uA"][t]) == bi else ohB
                            nc.tensor.matmul(
                                out=ps2[:],
                                lhsT=oh[:, t * P:(t + 1) * P],
                                rhs=m2[:, t * 4:(t + 1) * 4],
                                start=(t == t0), stop=(t == t1))
                        rd2 = pcb.tile([P, 1], f32, tag="rd2")
                        nc.vector.reciprocal(out=rd2[:], in_=ps2[:, 2:3])
                        v = pcb.tile([P, CLS], f32, tag="v")
                        nc.vector.tensor_scalar_mul(out=v[:], in0=ps2[:, 0:CLS],
                                                    scalar1=rd2[:, 0:1])
                        nc.vector.tensor_tensor(
                            out=vcol[:, b * CLS:(b + 1) * CLS],
                            in0=v[:], in1=b2s[:], op=mybir.AluOpType.add)

            # ---------------- batched log_softmax + output -------------------
            with tc.tile_pool(name="pf", bufs=1) as pf:
                mx = pf.tile([P, NB], f32)
                nc.vector.tensor_reduce(
                    out=mx[:], in_=vcol[:].rearrange("p (b c) -> p b c", b=NB),
                    axis=mybir.AxisListType.X, op=mybir.AluOpType.max)
                u = pf.tile([P, NB * CLS], f32)
                nc.vector.tensor_tensor(
                    out=u[:].rearrange("p (b c) -> p b c", b=NB),
                    in0=vcol[:].rearrange("p (b c) -> p b c", b=NB),
                    in1=_sub(mx[:], 0, [[1, NB], [0, CLS]]),
                    op=mybir.AluOpType.subtract)
                nc.scalar.activation(out=u[:], in_=u[:], func=EXP)
                sm = pf.tile([P, NB], f32)
                nc.vector.tensor_reduce(
                    out=sm[:], in_=u[:].rearrange("p (b c) -> p b c", b=NB),
                    axis=mybir.AxisListType.X, op=mybir.AluOpType.add)
                ls = pf.tile([P, NB], f32)
                nc.scalar.activation(out=ls[:], in_=sm[:], func=LN)
                nc.vector.tensor_tensor(out=ls[:], in0=ls[:], in1=mx[:],
                                        op=mybir.AluOpType.add)
                res = pf.tile([P, NB * CLS], f32)
                nc.vector.tensor_tensor(
                    out=res[:].rearrange("p (b c) -> p b c", b=NB),
                    in0=vcol[:].rearrange("p (b c) -> p b c", b=NB),
                    in1=_sub(ls[:], 0, [[1, NB], [0, CLS]]),
                    op=mybir.AluOpType.subtract)
                nc.sync.dma_start(out=out_d[:, :], in_=res[:])
    nc.finalize()
    return nc


def install_ntff_hook(so_path="/opt/axon/libaxon_pjrt.so"):
    import types
    import ctypes
    import contextlib
    import antenv

    if getattr(antenv, "axon_hooks", None) is not None:
        return
    lib = ctypes.CDLL(so_path)
    if not hasattr(lib, "axon_start_nrt_profile"):
        return
    lib.axon_start_nrt_profile.argtypes = [ctypes.POINTER(ctypes.c_int64),
                                           ctypes.c_size_t]
    lib.axon_start_nrt_profile.restype = ctypes.c_int64
    lib.axon_stop_nrt_profile.argtypes = [ctypes.c_char_p]
    lib.axon_stop_nrt_profile.restype = ctypes.c_int64

    @contextlib.contextmanager
    def _hook(output_dir, device_ids):
        import jax
        jax.devices()
        if device_ids:
            ids = (ctypes.c_int64 * len(device_ids))(*device_ids)
            rc = lib.axon_start_nrt_profile(ids, len(device_ids))
        else:
            rc = lib.axon_start_nrt_profile(None, 0)
        if rc != 0:
            raise RuntimeError(f"axon_start_nrt_profile rc={rc}")
        try:
            yield
        finally:
            n = lib.axon_stop_nrt_profile(str(output_dir).encode())
            print(f"ntff profile: {n} file(s) written to {output_dir}")

    mod = types.ModuleType("antenv.axon_hooks")
    _reg = [_hook]
    mod.set_axon_ntff_profile_hook = lambda h: _reg.__setitem__(0, h)
    mod.get_axon_ntff_profile_hook = lambda: _reg[0]
    sys.modules["antenv.axon_hooks"] = mod
    antenv.axon_hooks = mod


def run(inputs, cfg, trace=False, **kwargs):
    if trace:
        install_ntff_hook()
    in_maps, meta = prep(inputs, cfg)
    nc = build(meta)
    res = bass_utils.run_bass_kernel_spmd(
        nc, in_maps, core_ids=list(range(cfg["NC"])), trace=trace, **kwargs)
    NB, NPC, CLS = meta["NB"], meta["NPC"], meta["CLS"]
    outs = []
    for c in range(cfg["NC"]):
        r = np.asarray(res.results[c]["out"])          # [P, NB*CLS]
        r = r.reshape(P, NB, CLS).transpose(1, 0, 2).reshape(NB * P, CLS)
        outs.append(r[:NPC])
    return np.concatenate(outs, axis=0), res


# ----------------------------------------------------------------------------
# harness entry point
# ----------------------------------------------------------------------------

_CFG = dict(N=100000, F=165, H=4, C=64, CLS=2, NC=8)


def kernel(**inputs):
    """Full (unsharded) inputs -> full [N, 2] float32 log-softmax output.

    Shards edges by destination-node range across the 8 NeuronCores,
    compiles and runs the Bass/Tile kernel via run_bass_kernel_spmd,
    and concatenates the per-core output slices.
    """
    out, _ = run(inputs, _CFG, trace=False)
    return np.ascontiguousarray(out.astype(np.float32))
